# revision 1
# baseline (speedup 1.0000x reference)
"""Trainium2 Bass kernel for nn_MixFusionFeedForward (self-contained).

Data-parallel over the 16 video clips (2 per NeuronCore). Per clip:
  x[720,512] --DMA cast+transpose--> xT bf16 [512,720]
  "tconv": phase-decomposed stride-3 transposed conv == matmul1 + fold fused:
     img[(a,b)][ch, m, n] = sum_{s<S_a, t<S_b} (x @ w1[:,ch,a+3s,b+3t])[m-s,n-t]
     as 21 shifted matmuls accumulating in PSUM (bf16 in, fp32 accum).
  norm: imgn = img * invnorm + b1*invnorm  (DVE, PSUM -> SBUF bf16)
  unfold: X2[(phase,s,ch,t)][i,j] = imgn[phase][ch, i+s, j+t]  (SBUF->SBUF DMA)
  dwconv: depthwise 3x3/5x5 = diagonal 32x32 matmuls, 16 PE sub-array tiles
     concurrent (tile_position packing), taps accumulate in PSUM.
  gelu(+conv bias) on ScalarE evacuating PSUM -> g bf16
  mm2: out[l,:] = g.T @ w2r + b2 (bf16 matmul, fp32 accum)
"""
import sys
if '/opt/trn_rl_repo' not in sys.path:
    sys.path.insert(0, '/opt/trn_rl_repo')

import numpy as np
import ml_dtypes

D = 512
HD = 1960
NCH = 40
KH = KW = 7
HOUT, WOUT = 20, 36
L = HOUT * WOUT
T = 8
B = 2
NCLIP = B * T
NCORE = 8
CPC = NCLIP // NCORE
PM, PN = 22, 38
PMN = PM * PN
S = (3, 2, 2)
IH, IW = 60, 108
PH = PW = 3
PLH, PLW = 24, 40
PLANE = PLH * PLW
GROWS = 1024
NCHUNK = 16
BF16 = ml_dtypes.bfloat16

TCONV_MMS = [(s, t, a) for s in range(3) for t in range(3) for a in range(3)
             if s < S[a]]
assert len(TCONV_MMS) == 21


def _sem_rows():
    rows = []
    for half in (0, 1):
        cnt = 0
        for a in range(3):
            for b in range(3):
                for s in range(S[a]):
                    for ch in range(20):
                        for t in range(S[b]):
                            ki, kj = a + 3 * s, b + 3 * t
                            chfull = half * 20 + ch
                            rows.append(dict(half=half, a=a, b=b, s=s, ch=ch,
                                             t=t, cdw=chfull * 49 + ki * 7 + kj))
                            cnt += 1
        assert cnt == 980
        rows.extend([None] * (GROWS - 980))
    return rows


def _sem_to_phys(row):
    q, p = divmod(row, 128)
    r, o = divmod(p, 32)
    return q * 128 + 32 * ((r + q) % 4) + o


def build_consts(w1, b1, w3, b3, w5, b5, w2, b2):
    rows = _sem_rows()
    w1r = w1.reshape(D, NCH, KH, KW)
    b1r = b1.reshape(NCH, KH, KW)

    w1f = np.zeros((128, len(TCONV_MMS) * 4 * 120), np.float32)
    for mi, (s, t, a) in enumerate(TCONV_MMS):
        tl = np.zeros((D, 120), np.float32)
        for b_ in range(3):
            if t >= S[b_]:
                continue
            for half in (0, 1):
                cols = 40 * b_ + 20 * half + np.arange(20)
                tl[:, cols] = w1r[:, half * 20:half * 20 + 20,
                                  a + 3 * s, b_ + 3 * t]
        for k in range(4):
            w1f[:, (mi * 4 + k) * 120:(mi * 4 + k + 1) * 120] = \
                tl[k * 128:(k + 1) * 128]
    w1f = w1f.astype(BF16)

    nr = np.zeros(3 * PM)
    ncv = np.zeros(3 * PN)
    for i in range(HOUT):
        nr[3 * i:3 * i + KH] += 1
    for j in range(WOUT):
        ncv[3 * j:3 * j + KW] += 1
    invn = np.zeros((3, 3, PM, PN), np.float32)
    for a in range(3):
        for b_ in range(3):
            r = 3 * np.arange(PM) + a
            c = 3 * np.arange(PN) + b_
            rv = (r >= PH) & (r <= IH + PH - 1)
            cv = (c >= PW) & (c <= IW + PW - 1)
            with np.errstate(divide='ignore'):
                iv = 1.0 / np.outer(nr[r], ncv[c])
            iv[~rv, :] = 0
            iv[:, ~cv] = 0
            invn[a, b_] = iv
    b1img = np.zeros((NCH, 3, 3, PM, PN), np.float32)
    for a in range(3):
        for b_ in range(3):
            for s in range(S[a]):
                for t in range(S[b_]):
                    v = b1r[:, a + 3 * s, b_ + 3 * t]
                    b1img[:, a, b_, s:s + HOUT, t:t + WOUT] += v[:, None, None]
    invn_sb = np.zeros((128, 3 * PMN), np.float32)
    b1n_sb = np.zeros((128, 3 * PMN), np.float32)
    for a in range(3):
        for b_ in range(3):
            for half in (0, 1):
                for ch in range(20):
                    p = 40 * b_ + 20 * half + ch
                    iv = invn[a, b_]
                    invn_sb[p, a * PMN:(a + 1) * PMN] = iv.ravel()
                    b1n_sb[p, a * PMN:(a + 1) * PMN] = \
                        (b1img[half * 20 + ch, a, b_] * iv).ravel()
    invn_sb = invn_sb.astype(BF16)
    b1n_sb = b1n_sb.astype(BF16)

    def dw_weight(row, du, dv, k, off):
        info = rows[row]
        if info is None:
            return 0.0
        w = w3[info['cdw'], 0] if k == 3 else w5[info['cdw'] - HD // 2, 0]
        return float(w[du + off, dv + off])

    dga = np.zeros((128, 8 * 9 * 32), np.float32)
    dgb = np.zeros((128, 8 * 25 * 32), np.float32)
    for q in range(8):
        for r in range(4):
            for o in range(32):
                rowa = q * 128 + 32 * r + o
                rowb = (8 + q) * 128 + 32 * r + o
                for uvi, (du, dv) in enumerate(
                        (du, dv) for du in (-1, 0, 1) for dv in (-1, 0, 1)):
                    dga[32 * r + o, (q * 9 + uvi) * 32 + o] = \
                        dw_weight(rowa, du, dv, 3, 1)
                for uvi, (du, dv) in enumerate(
                        (du, dv) for du in (-2, -1, 0, 1, 2)
                        for dv in (-2, -1, 0, 1, 2)):
                    dgb[32 * r + o, (q * 25 + uvi) * 32 + o] = \
                        dw_weight(rowb, du, dv, 5, 2)
    dga = dga.astype(BF16)
    dgb = dgb.astype(BF16)

    w2r = np.zeros((128, NCHUNK * D), np.float32)
    bconv = np.zeros((128, NCHUNK), np.float32)
    for row in range(2048):
        info = rows[row]
        phys = _sem_to_phys(row)
        q, p = divmod(phys, 128)
        if info is not None:
            w2r[p, q * D:(q + 1) * D] = w2[info['cdw']]
            cdw = info['cdw']
            bconv[p, q] = b3[cdw] if cdw < HD // 2 else b5[cdw - HD // 2]
    w2r = w2r.astype(BF16)
    b2rep = np.tile(b2[None, :], (128, 1)).astype(np.float32)

    return dict(w1f=w1f, invn=invn_sb, b1n=b1n_sb, dga=dga, dgb=dgb,
                w2r=w2r, bconv=bconv, b2rep=b2rep)


def _unfold_plan():
    plan = []
    for half in (0, 1):
        base = half * GROWS
        cnt = 0
        for a in range(3):
            for b in range(3):
                for s in range(S[a]):
                    plan.append((half, a, b, s, base + cnt, 20 * S[b]))
                    cnt += 20 * S[b]
    return plan


_UNFOLD_PLAN = _unfold_plan()

_BUILT = None


def _build():
    global _BUILT
    if _BUILT is not None:
        return _BUILT
    import concourse.bacc as bacc
    import concourse.tile as tile
    import concourse.mybir as mybir
    import bass_rust
    from contextlib import ExitStack

    dt = mybir.dt
    AF = mybir.ActivationFunctionType
    OP = mybir.AluOpType

    def view(ap2d, p0, pcnt, off, dims, pstep=1):
        """Arbitrary free-dim view of a [128, F] tile, partitions
        p0, p0+pstep, ... (pcnt of them)."""
        tp = ap2d[p0:p0 + 1, :]
        pitch = ap2d.ap[0][0]
        return bass_rust.AP(tp.tensor, tp.offset + off,
                            [[pitch * pstep, pcnt]]
                            + [[s, c] for s, c in dims])

    nc = bacc.Bacc("TRN2", target_bir_lowering=False, debug=False,
                   enable_asserts=False, num_devices=NCORE)

    x_d = nc.dram_tensor("x_in", [CPC * L, D], dt.float32,
                         kind="ExternalInput").ap()
    w1f_d = nc.dram_tensor("w1f", [128, 21 * 4 * 120], dt.bfloat16,
                           kind="ExternalInput").ap()
    invn_d = nc.dram_tensor("invn", [128, 3 * PMN], dt.bfloat16,
                            kind="ExternalInput").ap()
    b1n_d = nc.dram_tensor("b1n", [128, 3 * PMN], dt.bfloat16,
                           kind="ExternalInput").ap()
    dga_d = nc.dram_tensor("dga", [128, 8 * 9 * 32], dt.bfloat16,
                           kind="ExternalInput").ap()
    dgb_d = nc.dram_tensor("dgb", [128, 8 * 25 * 32], dt.bfloat16,
                           kind="ExternalInput").ap()
    w2r_d = nc.dram_tensor("w2r", [128, NCHUNK * D], dt.bfloat16,
                           kind="ExternalInput").ap()
    bconv_d = nc.dram_tensor("bconv", [128, NCHUNK], dt.float32,
                             kind="ExternalInput").ap()
    b2rep_d = nc.dram_tensor("b2rep", [128, D], dt.float32,
                             kind="ExternalInput").ap()
    out_d = nc.dram_tensor("y_out", [CPC * L, D], dt.float32,
                           kind="ExternalOutput").ap()

    with tile.TileContext(nc) as tc, ExitStack() as ctx:
        dram_pool = ctx.enter_context(
            tc.tile_pool(name="dram", bufs=1, space="DRAM"))
        xbf_d = dram_pool.tile([CPC * L, D], dt.bfloat16)

        consts = ctx.enter_context(tc.tile_pool(name="consts", bufs=1))

        def cload(nm, dram_ap, shape, dtype):
            t = consts.tile(shape, dtype, tag=nm, name=f"c_{nm}")
            nc.sync.dma_start(t[:, :], dram_ap[:, :])
            return t

        w1f = cload('w1f', w1f_d, [128, 21 * 4 * 120], dt.bfloat16)
        invn = cload('invn', invn_d, [128, 3 * PMN], dt.bfloat16)
        b1n = cload('b1n', b1n_d, [128, 3 * PMN], dt.bfloat16)
        dga = cload('dga', dga_d, [128, 8 * 9 * 32], dt.bfloat16)
        dgb = cload('dgb', dgb_d, [128, 8 * 25 * 32], dt.bfloat16)
        w2r = cload('w2r', w2r_d, [128, NCHUNK * D], dt.bfloat16)
        bconv = cload('bconv', bconv_d, [128, NCHUNK], dt.float32)
        b2rep = cload('b2rep', b2rep_d, [128, D], dt.float32)

        nc.gpsimd.dma_start(xbf_d[:, :], x_d[:, :])

        xt_pool = ctx.enter_context(tc.tile_pool(name="xt", bufs=2))
        imgn_pool = ctx.enter_context(tc.tile_pool(name="imgn", bufs=2))
        x2_pool = ctx.enter_context(tc.tile_pool(name="x2", bufs=2))
        g_pool = ctx.enter_context(tc.tile_pool(name="g", bufs=1))
        osb_pool = ctx.enter_context(tc.tile_pool(name="osb", bufs=2))

        for clip in range(CPC):
            # ---------- xT ----------
            xt = xt_pool.tile([128, 4 * L], dt.bfloat16, tag="xt")
            for kc in range(4):
                nc.sync.dma_start(
                    xt[:, kc * L:(kc + 1) * L],
                    xbf_d[clip * L:(clip + 1) * L, kc * 128:(kc + 1) * 128],
                    transpose=True)

            # ---------- tconv ----------
            imgn = imgn_pool.tile([128, 3 * PMN], dt.bfloat16, tag="imgn")
            with tc.tile_pool(name="tcps", bufs=2, space="PSUM") as tps:
                for jh in range(2):
                    ps = [tps.tile([128, 512], dt.float32, tag=f"tc{a}",
                                   name=f"tcps{a}")
                          for a in range(3)]
                    nmm = {a: sum(1 for (_, _, aa) in TCONV_MMS if aa == a)
                           for a in range(3)}
                    cnt = {0: 0, 1: 0, 2: 0}
                    for mi, (s, t, a) in enumerate(TCONV_MMS):
                        cnt[a] += 1
                        if jh == 0:
                            wjd, loc_off, src_off = 19 - t, s * 19 + t, 0
                        else:
                            wjd, loc_off, src_off = 17 + t, s * 19, 19 - t
                        out_ap = view(ps[a], 0, 120, loc_off,
                                      [(19, HOUT), (1, wjd)])
                        for k in range(4):
                            rhs = view(xt, 0, 128, k * L + src_off,
                                       [(WOUT, HOUT), (1, wjd)])
                            lhsT = w1f[:, (mi * 4 + k) * 120:
                                       (mi * 4 + k + 1) * 120]
                            nc.tensor.matmul(out_ap, lhsT, rhs,
                                             start=(cnt[a] == 1 and k == 0),
                                             stop=(cnt[a] == nmm[a]
                                                   and k == 3))
                    for a in range(3):
                        dims = [(19, PM), (1, 19)]
                        ps_ap = view(ps[a], 0, 120, 0, dims)
                        im_ap = view(imgn, 0, 120, a * PMN + jh * 19,
                                     [(PN, PM), (1, 19)])
                        iv_ap = view(invn, 0, 120, a * PMN + jh * 19,
                                     [(PN, PM), (1, 19)])
                        b1_ap = view(b1n, 0, 120, a * PMN + jh * 19,
                                     [(PN, PM), (1, 19)])
                        nc.vector.tensor_tensor(im_ap, ps_ap, iv_ap, OP.mult)
                        nc.vector.tensor_tensor(im_ap, im_ap, b1_ap, OP.add)

            # ---------- unfold ----------
            x2a = x2_pool.tile([128, 8 * PLANE], dt.bfloat16, tag="x2a")
            x2b = x2_pool.tile([128, 8 * PLANE], dt.bfloat16, tag="x2b")
            for x2t in (x2a, x2b):
                nc.vector.memset(
                    view(x2t, 0, 128, 0, [(PLANE, 8), (1, 2 * PLW + 2)]), 0)
                nc.vector.memset(
                    view(x2t, 0, 128, 22 * PLW - 2,
                         [(PLANE, 8), (1, 2 * PLW + 2)]), 0)
                nc.vector.memset(
                    view(x2t, 0, 128, 2 * PLW + 38,
                         [(PLANE, 8), (PLW, 20), (1, 4)]), 0)
                # pad rows 980..1023 (chunk 7, partitions 84..127): zero the
                # whole plane so zero-diag matmuls see 0, not NaN garbage.
                # (32-aligned partition base required; unfold rewrites the
                # real rows 960..979 afterwards.)
                nc.vector.memset(x2t[64:128, 7 * PLANE:8 * PLANE], 0)

            for (half, a, b, s, r0g, n) in _UNFOLD_PLAN:
                sb = S[b]
                x2t = x2a if half == 0 else x2b
                pbase = 40 * b + 20 * half
                r0 = r0g % GROWS
                for t in range(sb):
                    # rows r0 + ch*sb + t for ch in 0..19; split where the
                    # 128-row chunk index changes
                    ch0 = 0
                    while ch0 < 20:
                        q = (r0 + t + ch0 * sb) // 128
                        ch1 = ch0
                        while ch1 < 20 and (r0 + t + ch1 * sb) // 128 == q:
                            ch1 += 1
                        nch_ = ch1 - ch0
                        p0 = (r0 + t + ch0 * sb) - q * 128
                        ssrc = view(imgn, pbase + ch0, nch_,
                                    a * PMN + s * PN + t,
                                    [(PN, HOUT), (1, WOUT)])
                        dst = view(x2t, p0, nch_,
                                   q * PLANE + 2 * PLW + 2,
                                   [(PLW, HOUT), (1, WOUT)], pstep=sb)
                        nc.sync.dma_start(dst, ssrc)
                        ch0 = ch1

            # ---------- dwconv ----------
            g = g_pool.tile([128, NCHUNK * L], dt.bfloat16, tag="g")
            with tc.tile_pool(name="dwps", bufs=2, space="PSUM") as dps:
                for grp in range(2):
                    x2t = x2a if grp == 0 else x2b
                    dg = dga if grp == 0 else dgb
                    nuv = 9 if grp == 0 else 25
                    ko = 1 if grp == 0 else 2
                    uvs = [(du, dv) for du in range(-ko, ko + 1)
                           for dv in range(-ko, ko + 1)]
                    for pg in range(2):
                        for jh in range(2):
                            ps = dps.tile([128, 4 * 512], dt.float32,
                                          tag="dw")
                            for uvi, (du, dv) in enumerate(uvs):
                                for ql in range(4):
                                    q = 4 * pg + ql
                                    for r in range(4):
                                        c = (r + q + 8 * grp) % 4
                                        lhsT = dg[32 * r:32 * r + 32,
                                                  (q * nuv + uvi) * 32:
                                                  (q * nuv + uvi + 1) * 32]
                                        rhs = view(
                                            x2t, 32 * r, 32,
                                            q * PLANE + (2 + du) * PLW
                                            + 2 + dv + jh * 18,
                                            [(PLW, HOUT), (1, 18)])
                                        out = view(ps, 32 * c, 32, ql * 512,
                                                   [(18, HOUT), (1, 18)])
                                        nc.tensor.matmul(
                                            out, lhsT, rhs,
                                            start=(uvi == 0),
                                            stop=(uvi == nuv - 1),
                                            tile_position=(32 * r, 32 * c))
                            for ql in range(4):
                                gq = 8 * grp + 4 * pg + ql
                                g_ap = view(g, 0, 128, gq * L + jh * 18,
                                            [(WOUT, HOUT), (1, 18)])
                                ps_ap = view(ps, 0, 128, ql * 512,
                                             [(18, HOUT), (1, 18)])
                                nc.scalar.activation(
                                    g_ap, ps_ap, AF.Gelu,
                                    bias=bconv[:, gq:gq + 1], scale=1.0)

            # ---------- mm2 ----------
            with tc.tile_pool(name="mmps", bufs=2, space="PSUM") as mps:
                for mt in range(6):
                    pso = mps.tile([128, D], dt.float32, tag="mm2")
                    for kc in range(NCHUNK):
                        lhsT = g[:, kc * L + mt * 120:kc * L + mt * 120 + 120]
                        rhs = w2r[:, kc * D:(kc + 1) * D]
                        nc.tensor.matmul(pso[0:120, :], lhsT, rhs,
                                         start=(kc == 0),
                                         stop=(kc == NCHUNK - 1))
                    osb = osb_pool.tile([128, D], dt.float32, tag="osb")
                    nc.vector.tensor_tensor(osb[0:120, :], pso[0:120, :],
                                            b2rep[0:120, :], OP.add)
                    nc.sync.dma_start(
                        out_d[clip * L + mt * 120:
                              clip * L + mt * 120 + 120, :],
                        osb[0:120, :])

    nc.compile()
    _BUILT = nc
    return nc


def kernel(**inputs):
    x = np.asarray(inputs['x'], np.float32)
    consts = build_consts(
        np.asarray(inputs['w1'], np.float32),
        np.asarray(inputs['b1'], np.float32),
        np.asarray(inputs['w3'], np.float32),
        np.asarray(inputs['b3'], np.float32),
        np.asarray(inputs['w5'], np.float32),
        np.asarray(inputs['b5'], np.float32),
        np.asarray(inputs['w2'], np.float32),
        np.asarray(inputs['b2'], np.float32))
    nc = _build()
    from concourse.bass_utils import run_bass_kernel_spmd

    xf = x.reshape(NCLIP, L, D)
    in_maps = []
    for core in range(NCORE):
        m = {k: consts[k] for k in consts}
        m['x_in'] = np.ascontiguousarray(
            xf[core * CPC:(core + 1) * CPC].reshape(CPC * L, D))
        in_maps.append(m)
    res = run_bass_kernel_spmd(nc, in_maps, core_ids=list(range(NCORE)))
    out = np.zeros((NCLIP, L, D), np.float32)
    for core in range(NCORE):
        out[core * CPC:(core + 1) * CPC] = \
            res.results[core]['y_out'].reshape(CPC, L, D)
    return out.reshape(B, T * L, D)



# revision 6
# speedup vs baseline: 1.4665x; 1.4665x over previous
"""Trainium2 Bass kernel for nn_MixFusionFeedForward (self-contained).

Data-parallel over 16 video clips (2 per NeuronCore). Per clip:
  x[720,512] --host bf16 cast + DMA transpose--> xT [512,720]
  tconv: phase-decomposed stride-3 transposed conv == matmul1 + fold fused:
     21 shifted matmuls accumulating in PSUM (bf16 in, fp32 accum), M=128
     (padded from 120) so the compiler's fast-weight-load (FWL) engages.
  norm: imgn = psum * invnorm + b1*invnorm  (DVE, PSUM -> SBUF bf16)
  unfold: x2 planes are pitch-38 whole-row copies of shifted imgn windows
     (contiguous 1520B runs per partition, ~25 DMAs/clip). Conv zero-padding
     comes from memset pad columns 36/37 of every plane: column wraps through
     the pitch land on a neighbor row's zeroed pads.
  dwconv: depthwise 3x3/5x5 = diagonal 32x32 matmuls, 16 PE sub-array tiles,
     both clips in one matmul via a 3D rhs AP (rows split 7/7/6 to fit the
     512-fp32 PSUM bank), taps row-clipped per du, center tap first.
  gelu(+conv bias) on ScalarE evacuating PSUM -> g bf16
  mm2: out[l,:] = g.T @ w2r + b2 (bf16 matmul, fp32 accum), M=128 tiles.
"""
import sys
if '/opt/trn_rl_repo' not in sys.path:
    sys.path.insert(0, '/opt/trn_rl_repo')

import numpy as np
import ml_dtypes

D = 512
HD = 1960
NCH = 40
KH = KW = 7
HOUT, WOUT = 20, 36
L = HOUT * WOUT
T = 8
B = 2
NCLIP = B * T
NCORE = 8
CPC = NCLIP // NCORE
PM, PN = 22, 38
PMN = PM * PN
S = (3, 2, 2)
IH, IW = 60, 108
PH = PW = 3
PLANE = HOUT * PN          # 760: 20 rows x 38 (pitch 38, cols 36/37 = pads)
GROWS = 1024
NCHUNK = 16
MT_W = [128, 128, 128, 128, 128, 80]   # mm2 M tiling of 720 positions
RG = [(0, 7), (7, 14), (14, 20)]       # dwconv row groups (2 clips x rg x 36 <= 512)
BF16 = ml_dtypes.bfloat16

TCONV_MMS = [(s, t, a) for a in range(3) for s in range(3) for t in range(3)
             if s < S[a]]
assert len(TCONV_MMS) == 21

UVS3 = [(0, 0)] + [(du, dv) for du in (-1, 0, 1) for dv in (-1, 0, 1)
                   if (du, dv) != (0, 0)]
UVS5 = [(0, 0)] + [(du, dv) for du in (-2, -1, 0, 1, 2)
                   for dv in (-2, -1, 0, 1, 2) if (du, dv) != (0, 0)]


def _sem_rows():
    rows = []
    for half in (0, 1):
        cnt = 0
        for a in range(3):
            for b in range(3):
                for s in range(S[a]):
                    for ch in range(20):
                        for t in range(S[b]):
                            ki, kj = a + 3 * s, b + 3 * t
                            chfull = half * 20 + ch
                            rows.append(dict(half=half, a=a, b=b, s=s, ch=ch,
                                             t=t, cdw=chfull * 49 + ki * 7 + kj))
                            cnt += 1
        assert cnt == 980
        rows.extend([None] * (GROWS - 980))
    return rows


def _sem_to_phys(row):
    q, p = divmod(row, 128)
    r, o = divmod(p, 32)
    return q * 128 + 32 * ((r + q) % 4) + o


def build_consts(w1, b1, w3, b3, w5, b5, w2, b2):
    rows = _sem_rows()
    w1r = w1.reshape(D, NCH, KH, KW)
    b1r = b1.reshape(NCH, KH, KW)

    w1f = np.zeros((128, len(TCONV_MMS) * 4 * 128), np.float32)
    for mi, (s, t, a) in enumerate(TCONV_MMS):
        tl = np.zeros((D, 128), np.float32)
        for b_ in range(3):
            if t >= S[b_]:
                continue
            for half in (0, 1):
                cols = 40 * b_ + 20 * half + np.arange(20)
                tl[:, cols] = w1r[:, half * 20:half * 20 + 20,
                                  a + 3 * s, b_ + 3 * t]
        for k in range(4):
            w1f[:, (mi * 4 + k) * 128:(mi * 4 + k + 1) * 128] = \
                tl[k * 128:(k + 1) * 128]
    w1f = w1f.astype(BF16)

    nr = np.zeros(3 * PM)
    ncv = np.zeros(3 * PN)
    for i in range(HOUT):
        nr[3 * i:3 * i + KH] += 1
    for j in range(WOUT):
        ncv[3 * j:3 * j + KW] += 1
    invn = np.zeros((3, 3, PM, PN), np.float32)
    for a in range(3):
        for b_ in range(3):
            r = 3 * np.arange(PM) + a
            c = 3 * np.arange(PN) + b_
            rv = (r >= PH) & (r <= IH + PH - 1)
            cv = (c >= PW) & (c <= IW + PW - 1)
            with np.errstate(divide='ignore'):
                iv = 1.0 / np.outer(nr[r], ncv[c])
            iv[~rv, :] = 0
            iv[:, ~cv] = 0
            invn[a, b_] = iv
    b1img = np.zeros((NCH, 3, 3, PM, PN), np.float32)
    for a in range(3):
        for b_ in range(3):
            for s in range(S[a]):
                for t in range(S[b_]):
                    v = b1r[:, a + 3 * s, b_ + 3 * t]
                    b1img[:, a, b_, s:s + HOUT, t:t + WOUT] += v[:, None, None]
    invn_sb = np.zeros((128, 3 * PMN), np.float32)
    b1n_sb = np.zeros((128, 3 * PMN), np.float32)
    for a in range(3):
        for b_ in range(3):
            for half in (0, 1):
                for ch in range(20):
                    p = 40 * b_ + 20 * half + ch
                    iv = invn[a, b_]
                    invn_sb[p, a * PMN:(a + 1) * PMN] = iv.ravel()
                    b1n_sb[p, a * PMN:(a + 1) * PMN] = \
                        (b1img[half * 20 + ch, a, b_] * iv).ravel()
    invn_sb = invn_sb.astype(BF16)
    b1n_sb = b1n_sb.astype(BF16)

    def dw_weight(row, du, dv, k, off):
        info = rows[row]
        if info is None:
            return 0.0
        w = w3[info['cdw'], 0] if k == 3 else w5[info['cdw'] - HD // 2, 0]
        return float(w[du + off, dv + off])

    dga = np.zeros((128, 8 * 9 * 32), np.float32)
    dgb = np.zeros((128, 8 * 25 * 32), np.float32)
    for q in range(8):
        for r in range(4):
            for o in range(32):
                rowa = q * 128 + 32 * r + o
                rowb = (8 + q) * 128 + 32 * r + o
                for uvi, (du, dv) in enumerate(UVS3):
                    dga[32 * r + o, (q * 9 + uvi) * 32 + o] = \
                        dw_weight(rowa, du, dv, 3, 1)
                for uvi, (du, dv) in enumerate(UVS5):
                    dgb[32 * r + o, (q * 25 + uvi) * 32 + o] = \
                        dw_weight(rowb, du, dv, 5, 2)
    dga = dga.astype(BF16)
    dgb = dgb.astype(BF16)

    w2r = np.zeros((128, NCHUNK * D), np.float32)
    bconv = np.zeros((128, NCHUNK), np.float32)
    for row in range(2048):
        info = rows[row]
        phys = _sem_to_phys(row)
        q, p = divmod(phys, 128)
        if info is not None:
            w2r[p, q * D:(q + 1) * D] = w2[info['cdw']]
            cdw = info['cdw']
            bconv[p, q] = b3[cdw] if cdw < HD // 2 else b5[cdw - HD // 2]
    w2r = w2r.astype(BF16)
    b2rep = np.tile(b2[None, :], (128, 1)).astype(np.float32)

    return dict(w1f=w1f, invn=invn_sb, b1n=b1n_sb, dga=dga, dgb=dgb,
                w2r=w2r, bconv=bconv, b2rep=b2rep)


def _unfold_plan():
    plan = []
    for half in (0, 1):
        base = half * GROWS
        cnt = 0
        for a in range(3):
            for b in range(3):
                for s in range(S[a]):
                    plan.append((half, a, b, s, base + cnt, 20 * S[b]))
                    cnt += 20 * S[b]
    return plan


_UNFOLD_PLAN = _unfold_plan()

_BUILT = None


def _build():
    global _BUILT
    if _BUILT is not None:
        return _BUILT
    import concourse.bacc as bacc
    import concourse.tile as tile
    import concourse.mybir as mybir
    import bass_rust
    from contextlib import ExitStack

    dt = mybir.dt
    AF = mybir.ActivationFunctionType
    OP = mybir.AluOpType

    def view(ap2d, p0, pcnt, off, dims, pstep=1):
        """Arbitrary free-dim view of a [128, F] tile, partitions
        p0, p0+pstep, ... (pcnt of them)."""
        tp = ap2d[p0:p0 + 1, :]
        pitch = ap2d.ap[0][0]
        return bass_rust.AP(tp.tensor, tp.offset + off,
                            [[pitch * pstep, pcnt]]
                            + [[s, c] for s, c in dims])

    nc = bacc.Bacc("TRN2", target_bir_lowering=False, debug=False,
                   enable_asserts=False, num_devices=NCORE)

    x_d = nc.dram_tensor("x_in", [CPC * L, D], dt.bfloat16,
                         kind="ExternalInput").ap()
    w1f_d = nc.dram_tensor("w1f", [128, 21 * 4 * 128], dt.bfloat16,
                           kind="ExternalInput").ap()
    invn_d = nc.dram_tensor("invn", [128, 3 * PMN], dt.bfloat16,
                            kind="ExternalInput").ap()
    b1n_d = nc.dram_tensor("b1n", [128, 3 * PMN], dt.bfloat16,
                           kind="ExternalInput").ap()
    dga_d = nc.dram_tensor("dga", [128, 8 * 9 * 32], dt.bfloat16,
                           kind="ExternalInput").ap()
    dgb_d = nc.dram_tensor("dgb", [128, 8 * 25 * 32], dt.bfloat16,
                           kind="ExternalInput").ap()
    w2r_d = nc.dram_tensor("w2r", [128, NCHUNK * D], dt.bfloat16,
                           kind="ExternalInput").ap()
    bconv_d = nc.dram_tensor("bconv", [128, NCHUNK], dt.float32,
                             kind="ExternalInput").ap()
    b2rep_d = nc.dram_tensor("b2rep", [128, D], dt.float32,
                             kind="ExternalInput").ap()
    out_d = nc.dram_tensor("y_out", [CPC * L, D], dt.float32,
                           kind="ExternalOutput").ap()

    X2CLIP = NCHUNK * PLANE          # per-clip x2 span (16 planes of 760)
    X2OFF = 2                        # front slack (zeroed; wrap target)

    with tile.TileContext(nc) as tc, ExitStack() as ctx:
        consts = ctx.enter_context(tc.tile_pool(name="consts", bufs=1))

        def cload(nm, dram_ap, shape, dtype):
            t = consts.tile(shape, dtype, tag=nm, name=f"c_{nm}")
            nc.sync.dma_start(t[:, :], dram_ap[:, :])
            return t

        w1f = cload('w1f', w1f_d, [128, 21 * 4 * 128], dt.bfloat16)

        xt_pool = ctx.enter_context(tc.tile_pool(name="xt", bufs=2))
        imgn_pool = ctx.enter_context(tc.tile_pool(name="imgn", bufs=2))
        x2_pool = ctx.enter_context(tc.tile_pool(name="x2", bufs=1))
        g_pool = ctx.enter_context(tc.tile_pool(name="g", bufs=1))
        osb_pool = ctx.enter_context(tc.tile_pool(name="osb", bufs=2))

        xts = []
        for clip in range(CPC):
            xt = xt_pool.tile([128, 4 * L], dt.bfloat16, tag="xt")
            for kc in range(4):
                nc.sync.dma_start(
                    xt[:, kc * L:(kc + 1) * L],
                    x_d[clip * L:(clip + 1) * L, kc * 128:(kc + 1) * 128],
                    transpose=True)
            xts.append(xt)

        invn = cload('invn', invn_d, [128, 3 * PMN], dt.bfloat16)
        b1n = cload('b1n', b1n_d, [128, 3 * PMN], dt.bfloat16)
        dga = cload('dga', dga_d, [128, 8 * 9 * 32], dt.bfloat16)
        dgb = cload('dgb', dgb_d, [128, 8 * 25 * 32], dt.bfloat16)
        w2r = cload('w2r', w2r_d, [128, NCHUNK * D], dt.bfloat16)
        bconv = cload('bconv', bconv_d, [128, NCHUNK], dt.float32)
        b2rep = cload('b2rep', b2rep_d, [128, D], dt.float32)

        x2 = x2_pool.tile([128, X2OFF + CPC * X2CLIP], dt.bfloat16, tag="x2")
        nc.vector.memset(x2[:, 0:X2OFF], 0)

        tcps_ctx = tc.tile_pool(name="tcps", bufs=2, space="PSUM")
        tcps_pool = tcps_ctx.__enter__()

        imgns = []
        for clip in range(CPC):
            # pad rows (980..1023 per half): zero whole planes so zero-diag
            # matmuls see 0. 32-aligned partition base; unfold rewrites the
            # real rows 960..979 afterwards.
            for half in (0, 1):
                nc.vector.memset(
                    view(x2, 64, 64,
                         X2OFF + clip * X2CLIP + (half * 8 + 7) * PLANE,
                         [(1, PLANE)]), 0)

            imgn = imgn_pool.tile([128, 3 * PMN + 16], dt.bfloat16,
                                  tag="imgn")
            imgns.append(imgn)
            xt = xts[clip]

            # ---------- tconv (a-major so norms/unfolds start early) ------
            for jh in range(2):
                for a in range(3):
                    taps = [(mi, s, t) for mi, (s, t, aa) in
                            enumerate(TCONV_MMS) if aa == a]
                    ps = tcps_pool.tile([128, 512], dt.float32, tag=f"tc{a}",
                                        name=f"tcps{a}")
                    nmm = len(taps) * 4
                    cnt = 0
                    for (mi, s, t) in taps:
                        if jh == 0:
                            wjd, loc_off, src_off = 19 - t, s * 19 + t, 0
                        else:
                            wjd, loc_off, src_off = 17 + t, s * 19, 19 - t
                        for k in range(4):
                            cnt += 1
                            rhs = view(xt, 0, 128, k * L + src_off,
                                       [(WOUT, HOUT), (1, wjd)])
                            lhsT = w1f[:, (mi * 4 + k) * 128:
                                       (mi * 4 + k + 1) * 128]
                            out_ap = view(ps, 0, 128, loc_off,
                                          [(19, HOUT), (1, wjd)])
                            nc.tensor.matmul(out_ap, lhsT, rhs,
                                             start=(cnt == 1),
                                             stop=(cnt == nmm))
                    # norm for this (a, jh)
                    dims = [(19, PM), (1, 19)]
                    ps_ap = view(ps, 0, 120, 0, dims)
                    im_ap = view(imgn, 0, 120, a * PMN + jh * 19,
                                 [(PN, PM), (1, 19)])
                    iv_ap = view(invn, 0, 120, a * PMN + jh * 19,
                                 [(PN, PM), (1, 19)])
                    b1_ap = view(b1n, 0, 120, a * PMN + jh * 19,
                                 [(PN, PM), (1, 19)])
                    nc.vector.tensor_tensor(im_ap, ps_ap, iv_ap, OP.mult)
                    nc.vector.tensor_tensor(im_ap, im_ap, b1_ap, OP.add)

                    if jh == 1:
                        # ---------- unfold for this phase a ----------
                        for (half, aa, b, s, r0g, nrw) in _UNFOLD_PLAN:
                            if aa != a:
                                continue
                            sb = S[b]
                            pbase = 40 * b + 20 * half
                            base_off = a * PMN + s * PN
                            r0 = (r0g % GROWS)
                            pos = 0
                            while pos < nrw:
                                row = r0 + pos
                                q = row // 128
                                lim = min(nrw, (q + 1) * 128 - r0)
                                n = lim - pos
                                ch0, t0 = divmod(pos, sb)
                                dsto = (X2OFF + clip * X2CLIP
                                        + (half * 8 + q) * PLANE)
                                p0 = row - q * 128
                                if t0 != 0 or n < sb:
                                    # partial t-run of one ch
                                    cnt_t = min(sb - t0, n)
                                    src = view(imgn, pbase + ch0, 1,
                                               base_off + t0,
                                               [(1, cnt_t), (1, PLANE)])
                                    dst = view(x2, p0, cnt_t, dsto,
                                               [(1, PLANE)])
                                    nc.sync.dma_start(dst, src)
                                    pos += cnt_t
                                else:
                                    nch_ = n // sb
                                    src = view(imgn, pbase + ch0, nch_,
                                               base_off,
                                               [(1, sb), (1, PLANE)])
                                    dst = view(x2, p0, nch_ * sb, dsto,
                                               [(1, PLANE)])
                                    nc.sync.dma_start(dst, src)
                                    pos += nch_ * sb
            # pad cols 36/37 of every plane of this clip -> conv zero-pad
            nc.vector.memset(
                view(x2, 0, 128, X2OFF + clip * X2CLIP + WOUT,
                     [(PLANE, NCHUNK), (PN, HOUT), (1, 2)]), 0)

        tcps_ctx.__exit__(None, None, None)

        # ---------- dwconv (both clips per matmul) ----------
        gs = [g_pool.tile([128, NCHUNK * L], dt.bfloat16, tag=f"g{c}",
                          name=f"g{c}")
              for c in range(CPC)]
        with tc.tile_pool(name="dwps", bufs=2, space="PSUM") as dps:
            for grp in range(2):
                dg = dga if grp == 0 else dgb
                uvs = UVS3 if grp == 0 else UVS5
                nuv = len(uvs)
                for pg in range(2):
                    for (rg0, rg1) in RG:
                        nrg = rg1 - rg0
                        ps = dps.tile([128, 4 * 512], dt.float32, tag="dw")
                        for uvi, (du, dv) in enumerate(uvs):
                            i0 = max(rg0, -du)
                            i1 = min(rg1, HOUT - max(0, du))
                            ni = i1 - i0
                            for ql in range(4):
                                q = 4 * pg + ql
                                for r in range(4):
                                    c = (r + q) % 4
                                    lhsT = dg[32 * r:32 * r + 32,
                                              (q * nuv + uvi) * 32:
                                              (q * nuv + uvi + 1) * 32]
                                    rhs = view(
                                        x2, 32 * r, 32,
                                        X2OFF + (grp * 8 + q) * PLANE
                                        + (i0 + du) * PN + dv,
                                        [(X2CLIP, CPC), (PN, ni), (1, WOUT)])
                                    out = view(
                                        ps, 32 * c, 32,
                                        ql * 512 + (i0 - rg0) * WOUT,
                                        [(nrg * WOUT, CPC), (WOUT, ni),
                                         (1, WOUT)])
                                    nc.tensor.matmul(
                                        out, lhsT, rhs,
                                        start=(uvi == 0),
                                        stop=(uvi == nuv - 1),
                                        tile_position=(32 * r, 32 * c))
                        for ql in range(4):
                            gq = 8 * grp + 4 * pg + ql
                            for clip in range(CPC):
                                ps_ap = view(ps, 0, 128,
                                             ql * 512 + clip * nrg * WOUT,
                                             [(WOUT, nrg), (1, WOUT)])
                                g_ap = view(gs[clip], 0, 128,
                                            gq * L + rg0 * WOUT,
                                            [(WOUT, nrg), (1, WOUT)])
                                nc.scalar.activation(
                                    g_ap, ps_ap, AF.Gelu,
                                    bias=bconv[:, gq:gq + 1], scale=1.0)

        # ---------- mm2 ----------
        with tc.tile_pool(name="mmps", bufs=2, space="PSUM") as mps:
            for clip in range(CPC):
                g = gs[clip]
                moff = 0
                for mt in range(6):
                    mw = MT_W[mt]
                    pso = mps.tile([128, D], dt.float32, tag="mm2")
                    for kc in range(NCHUNK):
                        lhsT = g[:, kc * L + moff:kc * L + moff + mw]
                        rhs = w2r[:, kc * D:(kc + 1) * D]
                        nc.tensor.matmul(pso[0:mw, :], lhsT, rhs,
                                         start=(kc == 0),
                                         stop=(kc == NCHUNK - 1))
                    osb = osb_pool.tile([128, D], dt.float32, tag="osb")
                    nc.vector.tensor_tensor(osb[0:mw, :], pso[0:mw, :],
                                            b2rep[0:mw, :], OP.add)
                    nc.sync.dma_start(
                        out_d[clip * L + moff:clip * L + moff + mw, :],
                        osb[0:mw, :])
                    moff += mw

    nc.compile()
    _BUILT = nc
    return nc


def make_in_maps(inputs):
    x = np.asarray(inputs['x'], np.float32)
    consts = build_consts(
        np.asarray(inputs['w1'], np.float32),
        np.asarray(inputs['b1'], np.float32),
        np.asarray(inputs['w3'], np.float32),
        np.asarray(inputs['b3'], np.float32),
        np.asarray(inputs['w5'], np.float32),
        np.asarray(inputs['b5'], np.float32),
        np.asarray(inputs['w2'], np.float32),
        np.asarray(inputs['b2'], np.float32))
    xf = x.reshape(NCLIP, L, D).astype(BF16)
    in_maps = []
    for core in range(NCORE):
        m = {k: consts[k] for k in consts}
        m['x_in'] = np.ascontiguousarray(
            xf[core * CPC:(core + 1) * CPC].reshape(CPC * L, D))
        in_maps.append(m)
    return in_maps


def kernel(**inputs):
    nc = _build()
    from concourse.bass_utils import run_bass_kernel_spmd

    in_maps = make_in_maps(inputs)
    res = run_bass_kernel_spmd(nc, in_maps, core_ids=list(range(NCORE)))
    out = np.zeros((NCLIP, L, D), np.float32)
    for core in range(NCORE):
        out[core * CPC:(core + 1) * CPC] = \
            res.results[core]['y_out'].reshape(CPC, L, D)
    return out.reshape(B, T * L, D)


# revision 11
# speedup vs baseline: 1.6202x; 1.1048x over previous
"""Trainium2 Bass kernel for nn_MixFusionFeedForward (self-contained).

Data-parallel over 16 video clips (2 per NeuronCore). Per clip:
  x[720,512] --host bf16 cast + DMA transpose--> xT [512,720]
  tconv: phase-decomposed stride-3 transposed conv == matmul1 + fold fused:
     21 shifted matmuls accumulating in PSUM (bf16 in, fp32 accum), M=128
     (padded from 120) so the compiler's fast-weight-load (FWL) engages.
  norm: imgn = psum * invnorm + b1*invnorm  (DVE, PSUM -> SBUF bf16)
  unfold: x2 planes are pitch-38 whole-row copies of shifted imgn windows
     (contiguous 1520B runs per partition, ~25 DMAs/clip). Conv zero-padding
     comes from memset pad columns 36/37 of every plane: column wraps through
     the pitch land on a neighbor row's zeroed pads.
  dwconv: depthwise 3x3/5x5 = diagonal 32x32 matmuls, 16 PE sub-array tiles,
     both clips in one matmul via a 3D rhs AP (rows split 7/7/6 to fit the
     512-fp32 PSUM bank), taps row-clipped per du, center tap first.
  gelu(+conv bias) on ScalarE evacuating PSUM -> g bf16
  mm2: out[l,:] = g.T @ w2r + b2 (bf16 matmul, fp32 accum), M=128 tiles.
"""
import sys
if '/opt/trn_rl_repo' not in sys.path:
    sys.path.insert(0, '/opt/trn_rl_repo')

import numpy as np
import ml_dtypes

D = 512
HD = 1960
NCH = 40
KH = KW = 7
HOUT, WOUT = 20, 36
L = HOUT * WOUT
T = 8
B = 2
NCLIP = B * T
NCORE = 8
CPC = NCLIP // NCORE
PM, PN = 22, 38
PMN = PM * PN
S = (3, 2, 2)
IH, IW = 60, 108
PH = PW = 3
PLANE = HOUT * PN          # 760: 20 rows x 38 (pitch 38, cols 36/37 = pads)
GROWS = 1024
NCHUNK = 16
MT_W = [128, 128, 128, 128, 128, 80]   # mm2 M tiling of 720 positions
RG = [(0, 7), (7, 14), (14, 20)]       # dwconv row groups (2 clips x rg x 36 <= 512)
BF16 = ml_dtypes.bfloat16

TCONV_MMS = [(s, t, a) for a in range(3) for s in range(3) for t in range(3)
             if s < S[a]]
assert len(TCONV_MMS) == 21

UVS3 = [(0, 0)] + [(du, dv) for du in (-1, 0, 1) for dv in (-1, 0, 1)
                   if (du, dv) != (0, 0)]
UVS5 = [(0, 0)] + [(du, dv) for du in (-2, -1, 0, 1, 2)
                   for dv in (-2, -1, 0, 1, 2) if (du, dv) != (0, 0)]


def _sem_rows():
    rows = []
    for half in (0, 1):
        cnt = 0
        for a in range(3):
            for b in range(3):
                for s in range(S[a]):
                    for ch in range(20):
                        for t in range(S[b]):
                            ki, kj = a + 3 * s, b + 3 * t
                            chfull = half * 20 + ch
                            rows.append(dict(half=half, a=a, b=b, s=s, ch=ch,
                                             t=t, cdw=chfull * 49 + ki * 7 + kj))
                            cnt += 1
        assert cnt == 980
        rows.extend([None] * (GROWS - 980))
    return rows


def _sem_to_phys(row):
    q, p = divmod(row, 128)
    r, o = divmod(p, 32)
    return q * 128 + 32 * ((r + q) % 4) + o


def build_consts(w1, b1, w3, b3, w5, b5, w2, b2):
    rows = _sem_rows()
    w1r = w1.reshape(D, NCH, KH, KW)
    b1r = b1.reshape(NCH, KH, KW)

    w1f = np.zeros((128, len(TCONV_MMS) * 4 * 128), np.float32)
    for mi, (s, t, a) in enumerate(TCONV_MMS):
        tl = np.zeros((D, 128), np.float32)
        for b_ in range(3):
            if t >= S[b_]:
                continue
            for half in (0, 1):
                cols = 40 * b_ + 20 * half + np.arange(20)
                tl[:, cols] = w1r[:, half * 20:half * 20 + 20,
                                  a + 3 * s, b_ + 3 * t]
        for k in range(4):
            w1f[:, (mi * 4 + k) * 128:(mi * 4 + k + 1) * 128] = \
                tl[k * 128:(k + 1) * 128]
    w1f = w1f.astype(BF16)

    nr = np.zeros(3 * PM)
    ncv = np.zeros(3 * PN)
    for i in range(HOUT):
        nr[3 * i:3 * i + KH] += 1
    for j in range(WOUT):
        ncv[3 * j:3 * j + KW] += 1
    invn = np.zeros((3, 3, PM, PN), np.float32)
    for a in range(3):
        for b_ in range(3):
            r = 3 * np.arange(PM) + a
            c = 3 * np.arange(PN) + b_
            rv = (r >= PH) & (r <= IH + PH - 1)
            cv = (c >= PW) & (c <= IW + PW - 1)
            with np.errstate(divide='ignore'):
                iv = 1.0 / np.outer(nr[r], ncv[c])
            iv[~rv, :] = 0
            iv[:, ~cv] = 0
            invn[a, b_] = iv
    b1img = np.zeros((NCH, 3, 3, PM, PN), np.float32)
    for a in range(3):
        for b_ in range(3):
            for s in range(S[a]):
                for t in range(S[b_]):
                    v = b1r[:, a + 3 * s, b_ + 3 * t]
                    b1img[:, a, b_, s:s + HOUT, t:t + WOUT] += v[:, None, None]
    invn_sb = np.zeros((128, 3 * PMN), np.float32)
    b1n_sb = np.zeros((128, 3 * PMN), np.float32)
    for a in range(3):
        for b_ in range(3):
            for half in (0, 1):
                for ch in range(20):
                    p = 40 * b_ + 20 * half + ch
                    iv = invn[a, b_]
                    invn_sb[p, a * PMN:(a + 1) * PMN] = iv.ravel()
                    b1n_sb[p, a * PMN:(a + 1) * PMN] = \
                        (b1img[half * 20 + ch, a, b_] * iv).ravel()
    invn_sb = invn_sb.astype(BF16)
    b1n_sb = b1n_sb.astype(BF16)

    def dw_weight(row, du, dv, k, off):
        info = rows[row]
        if info is None:
            return 0.0
        w = w3[info['cdw'], 0] if k == 3 else w5[info['cdw'] - HD // 2, 0]
        return float(w[du + off, dv + off])

    dga = np.zeros((128, 8 * 9 * 32), np.float32)
    dgb = np.zeros((128, 8 * 25 * 32), np.float32)
    for q in range(8):
        for r in range(4):
            for o in range(32):
                rowa = q * 128 + 32 * r + o
                rowb = (8 + q) * 128 + 32 * r + o
                for uvi, (du, dv) in enumerate(UVS3):
                    dga[32 * r + o, (q * 9 + uvi) * 32 + o] = \
                        dw_weight(rowa, du, dv, 3, 1)
                for uvi, (du, dv) in enumerate(UVS5):
                    dgb[32 * r + o, (q * 25 + uvi) * 32 + o] = \
                        dw_weight(rowb, du, dv, 5, 2)
    dga = dga.astype(BF16)
    dgb = dgb.astype(BF16)

    w2r = np.zeros((128, NCHUNK * D), np.float32)
    bconv = np.zeros((128, NCHUNK), np.float32)
    for row in range(2048):
        info = rows[row]
        phys = _sem_to_phys(row)
        q, p = divmod(phys, 128)
        if info is not None:
            w2r[p, q * D:(q + 1) * D] = w2[info['cdw']]
            cdw = info['cdw']
            bconv[p, q] = b3[cdw] if cdw < HD // 2 else b5[cdw - HD // 2]
    w2r = w2r.astype(BF16)
    b2rep = np.tile(b2[None, :], (128, 1)).astype(np.float32)

    return dict(w1f=w1f, invn=invn_sb, b1n=b1n_sb, dga=dga, dgb=dgb,
                w2r=w2r, bconv=bconv, b2rep=b2rep)


def _unfold_plan():
    plan = []
    for half in (0, 1):
        base = half * GROWS
        cnt = 0
        for a in range(3):
            for b in range(3):
                for s in range(S[a]):
                    plan.append((half, a, b, s, base + cnt, 20 * S[b]))
                    cnt += 20 * S[b]
    return plan


_UNFOLD_PLAN = _unfold_plan()

_BUILT = None


def _build():
    global _BUILT
    if _BUILT is not None:
        return _BUILT
    import concourse.bacc as bacc
    import concourse.tile as tile
    import concourse.mybir as mybir
    import bass_rust
    from contextlib import ExitStack

    dt = mybir.dt
    AF = mybir.ActivationFunctionType
    OP = mybir.AluOpType

    def view(ap2d, p0, pcnt, off, dims, pstep=1):
        """Arbitrary free-dim view of a [128, F] tile, partitions
        p0, p0+pstep, ... (pcnt of them)."""
        tp = ap2d[p0:p0 + 1, :]
        pitch = ap2d.ap[0][0]
        return bass_rust.AP(tp.tensor, tp.offset + off,
                            [[pitch * pstep, pcnt]]
                            + [[s, c] for s, c in dims])

    nc = bacc.Bacc("TRN2", target_bir_lowering=False, debug=False,
                   enable_asserts=False, num_devices=NCORE)

    x_d = nc.dram_tensor("x_in", [CPC * L, D], dt.bfloat16,
                         kind="ExternalInput").ap()
    w1f_d = nc.dram_tensor("w1f", [128, 21 * 4 * 128], dt.bfloat16,
                           kind="ExternalInput").ap()
    invn_d = nc.dram_tensor("invn", [128, 3 * PMN], dt.bfloat16,
                            kind="ExternalInput").ap()
    b1n_d = nc.dram_tensor("b1n", [128, 3 * PMN], dt.bfloat16,
                           kind="ExternalInput").ap()
    dga_d = nc.dram_tensor("dga", [128, 8 * 9 * 32], dt.bfloat16,
                           kind="ExternalInput").ap()
    dgb_d = nc.dram_tensor("dgb", [128, 8 * 25 * 32], dt.bfloat16,
                           kind="ExternalInput").ap()
    w2r_d = nc.dram_tensor("w2r", [128, NCHUNK * D], dt.bfloat16,
                           kind="ExternalInput").ap()
    bconv_d = nc.dram_tensor("bconv", [128, NCHUNK], dt.float32,
                             kind="ExternalInput").ap()
    b2rep_d = nc.dram_tensor("b2rep", [128, D], dt.float32,
                             kind="ExternalInput").ap()
    out_d = nc.dram_tensor("y_out", [CPC * L, D], dt.float32,
                           kind="ExternalOutput").ap()

    X2CLIP = NCHUNK * PLANE          # per-clip x2 span (16 planes of 760)
    X2OFF = 2                        # front slack (zeroed; wrap target)

    with tile.TileContext(nc) as tc, ExitStack() as ctx:
        consts = ctx.enter_context(tc.tile_pool(name="consts", bufs=1))

        def cload(nm, dram_ap, shape, dtype):
            t = consts.tile(shape, dtype, tag=nm, name=f"c_{nm}")
            nc.sync.dma_start(t[:, :], dram_ap[:, :])
            return t

        # split w1f into two tiles so the a=0 taps (first 9 blocks) can
        # start as soon as the first, smaller DMA lands
        NA0 = 9 * 4 * 128
        w1f_a = consts.tile([128, NA0], dt.bfloat16, tag="w1f_a",
                            name="w1f_a")
        nc.sync.dma_start(w1f_a[:, :], w1f_d[:, 0:NA0])

        xt_pool = ctx.enter_context(tc.tile_pool(name="xt", bufs=2))
        imgn_pool = ctx.enter_context(tc.tile_pool(name="imgn", bufs=2))
        x2_pool = ctx.enter_context(tc.tile_pool(name="x2", bufs=1))
        g_pool = ctx.enter_context(tc.tile_pool(name="g", bufs=1))
        osb_pool = ctx.enter_context(tc.tile_pool(name="osb", bufs=2))

        def load_xt(clip):
            xt = xt_pool.tile([128, 4 * L], dt.bfloat16, tag="xt",
                              name=f"xt{clip}")
            for kc in range(4):
                nc.sync.dma_start(
                    xt[:, kc * L:(kc + 1) * L],
                    x_d[clip * L:(clip + 1) * L, kc * 128:(kc + 1) * 128],
                    transpose=True)
            return xt

        xts = [load_xt(0)]

        w1f_b = consts.tile([128, 21 * 4 * 128 - NA0], dt.bfloat16,
                            tag="w1f_b", name="w1f_b")
        nc.sync.dma_start(w1f_b[:, :], w1f_d[:, NA0:21 * 4 * 128])

        xts.append(load_xt(1))

        def w1f_block(mi, k):
            col = (mi * 4 + k) * 128
            if col < NA0:
                return w1f_a[:, col:col + 128]
            return w1f_b[:, col - NA0:col - NA0 + 128]

        invn = cload('invn', invn_d, [128, 3 * PMN], dt.bfloat16)
        b1n = cload('b1n', b1n_d, [128, 3 * PMN], dt.bfloat16)
        dga = cload('dga', dga_d, [128, 8 * 9 * 32], dt.bfloat16)
        dgb = cload('dgb', dgb_d, [128, 8 * 25 * 32], dt.bfloat16)
        w2r = cload('w2r', w2r_d, [128, NCHUNK * D], dt.bfloat16)
        bconv = cload('bconv', bconv_d, [128, NCHUNK], dt.float32)
        b2rep = cload('b2rep', b2rep_d, [128, D], dt.float32)

        x2 = x2_pool.tile([128, X2OFF + CPC * X2CLIP], dt.bfloat16, tag="x2")
        nc.vector.memset(x2[:, 0:X2OFF], 0)

        tcps_ctx = tc.tile_pool(name="tcps", bufs=2, space="PSUM")
        tcps_pool = tcps_ctx.__enter__()

        imgns = []
        for clip in range(CPC):
            # pad rows (980..1023 per half): zero whole planes so zero-diag
            # matmuls see 0. 32-aligned partition base; unfold rewrites the
            # real rows 960..979 afterwards.
            for half in (0, 1):
                nc.vector.memset(
                    view(x2, 64, 64,
                         X2OFF + clip * X2CLIP + (half * 8 + 7) * PLANE,
                         [(1, PLANE)]), 0)

            imgn = imgn_pool.tile([128, 3 * PMN + 16], dt.bfloat16,
                                  tag="imgn")
            imgns.append(imgn)
            xt = xts[clip]

            # ---------- tconv (a-major so norms/unfolds start early) ------
            for jh in range(2):
                for a in range(3):
                    taps = [(mi, s, t) for mi, (s, t, aa) in
                            enumerate(TCONV_MMS) if aa == a]
                    ps = tcps_pool.tile([128, 512], dt.float32, tag=f"tc{a}",
                                        name=f"tcps{a}")
                    nmm = len(taps) * 4
                    cnt = 0
                    for (mi, s, t) in taps:
                        if jh == 0:
                            wjd, loc_off, src_off = 19 - t, s * 19 + t, 0
                        else:
                            wjd, loc_off, src_off = 17 + t, s * 19, 19 - t
                        for k in range(4):
                            cnt += 1
                            rhs = view(xt, 0, 128, k * L + src_off,
                                       [(WOUT, HOUT), (1, wjd)])
                            lhsT = w1f_block(mi, k)
                            out_ap = view(ps, 0, 128, loc_off,
                                          [(19, HOUT), (1, wjd)])
                            nc.tensor.matmul(out_ap, lhsT, rhs,
                                             start=(cnt == 1),
                                             stop=(cnt == nmm))
                    # norm for this (a, jh)
                    dims = [(19, PM), (1, 19)]
                    ps_ap = view(ps, 0, 120, 0, dims)
                    im_ap = view(imgn, 0, 120, a * PMN + jh * 19,
                                 [(PN, PM), (1, 19)])
                    iv_ap = view(invn, 0, 120, a * PMN + jh * 19,
                                 [(PN, PM), (1, 19)])
                    b1_ap = view(b1n, 0, 120, a * PMN + jh * 19,
                                 [(PN, PM), (1, 19)])
                    nc.vector.tensor_tensor(im_ap, ps_ap, iv_ap, OP.mult)
                    nc.vector.tensor_tensor(im_ap, im_ap, b1_ap, OP.add)

                    if jh == 1:
                        # ---------- unfold for this phase a ----------
                        for ei, (half, aa, b, s, r0g, nrw) in \
                                enumerate(_UNFOLD_PLAN):
                            if aa != a:
                                continue
                            # alternate HWDGE queues (sync / scalar)
                            eng = nc.sync if (ei % 2 == 0) else nc.scalar
                            sb = S[b]
                            pbase = 40 * b + 20 * half
                            base_off = a * PMN + s * PN
                            r0 = (r0g % GROWS)
                            pos = 0
                            while pos < nrw:
                                row = r0 + pos
                                q = row // 128
                                lim = min(nrw, (q + 1) * 128 - r0)
                                n = lim - pos
                                ch0, t0 = divmod(pos, sb)
                                dsto = (X2OFF + clip * X2CLIP
                                        + (half * 8 + q) * PLANE)
                                p0 = row - q * 128
                                if t0 != 0 or n < sb:
                                    # partial t-run of one ch
                                    cnt_t = min(sb - t0, n)
                                    src = view(imgn, pbase + ch0, 1,
                                               base_off + t0,
                                               [(1, cnt_t), (1, PLANE)])
                                    dst = view(x2, p0, cnt_t, dsto,
                                               [(1, PLANE)])
                                    eng.dma_start(dst, src)
                                    pos += cnt_t
                                else:
                                    nch_ = n // sb
                                    src = view(imgn, pbase + ch0, nch_,
                                               base_off,
                                               [(1, sb), (1, PLANE)])
                                    dst = view(x2, p0, nch_ * sb, dsto,
                                               [(1, PLANE)])
                                    eng.dma_start(dst, src)
                                    pos += nch_ * sb
            # pad cols 36/37 of every plane of this clip -> conv zero-pad
            nc.vector.memset(
                view(x2, 0, 128, X2OFF + clip * X2CLIP + WOUT,
                     [(PLANE, NCHUNK), (PN, HOUT), (1, 2)]), 0)

        tcps_ctx.__exit__(None, None, None)

        # ---------- dwconv (both clips per matmul) ----------
        gs = [g_pool.tile([128, NCHUNK * L], dt.bfloat16, tag=f"g{c}",
                          name=f"g{c}")
              for c in range(CPC)]
        with tc.tile_pool(name="dwps", bufs=2, space="PSUM") as dps:
            for grp in range(2):
                dg = dga if grp == 0 else dgb
                uvs = UVS3 if grp == 0 else UVS5
                nuv = len(uvs)
                for pg in range(2):
                    for (rg0, rg1) in RG:
                        nrg = rg1 - rg0
                        ps = dps.tile([128, 4 * 512], dt.float32, tag="dw")
                        for uvi, (du, dv) in enumerate(uvs):
                            i0 = max(rg0, -du)
                            i1 = min(rg1, HOUT - max(0, du))
                            ni = i1 - i0
                            for ql in range(4):
                                q = 4 * pg + ql
                                for r in range(4):
                                    c = (r + q) % 4
                                    lhsT = dg[32 * r:32 * r + 32,
                                              (q * nuv + uvi) * 32:
                                              (q * nuv + uvi + 1) * 32]
                                    rhs = view(
                                        x2, 32 * r, 32,
                                        X2OFF + (grp * 8 + q) * PLANE
                                        + (i0 + du) * PN + dv,
                                        [(X2CLIP, CPC), (PN, ni), (1, WOUT)])
                                    out = view(
                                        ps, 32 * c, 32,
                                        ql * 512 + (i0 - rg0) * WOUT,
                                        [(nrg * WOUT, CPC), (WOUT, ni),
                                         (1, WOUT)])
                                    nc.tensor.matmul(
                                        out, lhsT, rhs,
                                        start=(uvi == 0),
                                        stop=(uvi == nuv - 1),
                                        tile_position=(32 * r, 32 * c))
                        for ql in range(4):
                            gq = 8 * grp + 4 * pg + ql
                            for clip in range(CPC):
                                ps_ap = view(ps, 0, 128,
                                             ql * 512 + clip * nrg * WOUT,
                                             [(WOUT, nrg), (1, WOUT)])
                                g_ap = view(gs[clip], 0, 128,
                                            gq * L + rg0 * WOUT,
                                            [(WOUT, nrg), (1, WOUT)])
                                nc.scalar.activation(
                                    g_ap, ps_ap, AF.Gelu,
                                    bias=bconv[:, gq:gq + 1], scale=1.0)

        # ---------- mm2 ----------
        with tc.tile_pool(name="mmps", bufs=2, space="PSUM") as mps:
            for clip in range(CPC):
                g = gs[clip]
                moff = 0
                for mt in range(6):
                    mw = MT_W[mt]
                    pso = mps.tile([128, D], dt.float32, tag="mm2")
                    for kc in range(NCHUNK):
                        lhsT = g[:, kc * L + moff:kc * L + moff + mw]
                        rhs = w2r[:, kc * D:(kc + 1) * D]
                        nc.tensor.matmul(pso[0:mw, :], lhsT, rhs,
                                         start=(kc == 0),
                                         stop=(kc == NCHUNK - 1))
                    osb = osb_pool.tile([128, D], dt.float32, tag="osb")
                    nc.vector.tensor_tensor(osb[0:mw, :], pso[0:mw, :],
                                            b2rep[0:mw, :], OP.add)
                    nc.sync.dma_start(
                        out_d[clip * L + moff:clip * L + moff + mw, :],
                        osb[0:mw, :])
                    moff += mw

    nc.compile()
    _BUILT = nc
    return nc


def make_in_maps(inputs):
    x = np.asarray(inputs['x'], np.float32)
    consts = build_consts(
        np.asarray(inputs['w1'], np.float32),
        np.asarray(inputs['b1'], np.float32),
        np.asarray(inputs['w3'], np.float32),
        np.asarray(inputs['b3'], np.float32),
        np.asarray(inputs['w5'], np.float32),
        np.asarray(inputs['b5'], np.float32),
        np.asarray(inputs['w2'], np.float32),
        np.asarray(inputs['b2'], np.float32))
    xf = x.reshape(NCLIP, L, D).astype(BF16)
    in_maps = []
    for core in range(NCORE):
        m = {k: consts[k] for k in consts}
        m['x_in'] = np.ascontiguousarray(
            xf[core * CPC:(core + 1) * CPC].reshape(CPC * L, D))
        in_maps.append(m)
    return in_maps


def kernel(**inputs):
    nc = _build()
    from concourse.bass_utils import run_bass_kernel_spmd

    in_maps = make_in_maps(inputs)
    res = run_bass_kernel_spmd(nc, in_maps, core_ids=list(range(NCORE)))
    out = np.zeros((NCLIP, L, D), np.float32)
    for core in range(NCORE):
        out[core * CPC:(core + 1) * CPC] = \
            res.results[core]['y_out'].reshape(CPC, L, D)
    return out.reshape(B, T * L, D)
